# revision 1
# baseline (speedup 1.0000x reference)
"""Trainium2 Bass kernel for nn_MultiHeadAttention (B=4, S=2048, D=1024, H=16).

Sharding: 8 cores = (batch b in 0..3) x (head-half hb in 0..1).
Each core computes, for its batch b and its 8 heads:
  qT = (Q[b] @ W_q[hb].T).T        [512, S]   (features on partitions)
  kT = (K[b] @ W_k[hb].T).T        [512, S]
  v  =  V[b] @ W_v[hb].T           [S, 512]   (+ ones column per head)
  per head h: scoresT = kT_h x qT_h contracted over d_k -> [S(sk), S(sq)]
              expT = exp(scoresT/8)  (no max subtraction; scores are O(5))
              PV with ones-row gives unnormalized attn.T and the softmax
              denominator in the same PSUM accumulation (M=65 matmul)
  out_partial = attnT.T @ W_o[:, hb-slice].T      [S, 1024]
Host sums the two partial outputs per batch.

All matmuls run in float32r (TF32-like, full PE rate at free-dim >= 256).
"""

import sys

sys.path.insert(0, "/opt/trn_rl_repo")

from contextlib import ExitStack

import numpy as np

import concourse.bass as bass  # noqa: F401
import concourse.tile as tile
from concourse import bacc, mybir
from concourse.bass_utils import run_bass_kernel_spmd

F32 = mybir.dt.float32
FR = mybir.dt.float32r
BF = mybir.dt.bfloat16
EXP = mybir.ActivationFunctionType.Exp

D = 1024  # d_model
HD = 512  # head-dim slice per core (8 heads x 64)
DK = 64
NH = 8  # heads per core
P = 128


def build(S=2048):
    """Build the per-core Bass program (same program for all 8 cores)."""
    nc = bacc.Bacc(
        "TRN2",
        target_bir_lowering=False,
        debug=False,
        enable_asserts=False,
        num_devices=1,
    )

    xqt = nc.dram_tensor("xqt", [D, S], FR, kind="ExternalInput").ap()
    xkt = nc.dram_tensor("xkt", [D, S], FR, kind="ExternalInput").ap()
    xvt = nc.dram_tensor("xvt", [D, S], FR, kind="ExternalInput").ap()
    wqt = nc.dram_tensor("wqt", [D, HD], FR, kind="ExternalInput").ap()
    wkt = nc.dram_tensor("wkt", [D, HD], FR, kind="ExternalInput").ap()
    wvt = nc.dram_tensor("wvt", [D, HD], FR, kind="ExternalInput").ap()
    wot = nc.dram_tensor("wot", [HD, D], FR, kind="ExternalInput").ap()
    out = nc.dram_tensor("out", [S, D], F32, kind="ExternalOutput").ap()

    SC = 512  # phase-1 streaming chunk of S
    n_sc = S // SC
    n_sk = S // P  # sk tiles per head
    SQC = 1024 if S % 1024 == 0 else S  # sq chunk for phase 2
    n_sqc = S // SQC
    n_st = S // P  # phase-3 s tiles
    KO = D // P  # 8 contraction subtiles

    with tile.TileContext(nc) as tc, ExitStack() as ctx:
        pers = ctx.enter_context(tc.tile_pool(name="pers", bufs=1))
        wpool = ctx.enter_context(tc.tile_pool(name="wpool", bufs=2))
        xpool = ctx.enter_context(tc.tile_pool(name="xpool", bufs=2))
        epool = ctx.enter_context(tc.tile_pool(name="epool", bufs=2))
        rpool = ctx.enter_context(tc.tile_pool(name="rpool", bufs=2))
        bpool = ctx.enter_context(tc.tile_pool(name="bpool", bufs=2))
        tpool = ctx.enter_context(tc.tile_pool(name="tpool", bufs=2))
        opool = ctx.enter_context(tc.tile_pool(name="opool", bufs=4))

        # persistent intermediates
        qt = pers.tile([P, 4, S], BF)  # qT: [p, pair, s], feature = pair*128+p
        kt = pers.tile([P, 4, S], BF)
        va = pers.tile([P, n_sk, NH, DK + 1], BF)  # v_aug: [s%128, s//128, h, dv|1]
        at = pers.tile([P, 4, S], FR)  # attnT (normalized)

        nc.vector.memset(va[:, :, :, DK], 1.0)

        # ---------------- Phase 1: projections ----------------
        ps1_ctx = ExitStack()
        ps_small = ps1_ctx.enter_context(tc.tile_pool(name="ps_small", bufs=2, space="PSUM"))

        def qk_proj(w_dram, x_dram, dst):
            w = wpool.tile([P, KO, HD], FR, tag="w", name="w_qk")
            nc.sync.dma_start(w, w_dram.rearrange("(o p) m -> p o m", p=P))
            for c in range(n_sc):
                x = xpool.tile([P, KO, SC], FR, tag="x", name="x_qk")
                nc.sync.dma_start(
                    x, x_dram[:, c * SC : (c + 1) * SC].rearrange("(o p) s -> p o s", p=P)
                )
                for pr in range(4):
                    ps = ps_small.tile([P, 512], F32, tag="ps", name="ps_qk")
                    for k in range(KO):
                        nc.tensor.matmul(
                            ps[:, :SC],
                            lhsT=w[:, k, pr * P : (pr + 1) * P],
                            rhs=x[:, k, :],
                            start=(k == 0),
                            stop=(k == KO - 1),
                        )
                    nc.vector.tensor_copy(dst[:, pr, c * SC : (c + 1) * SC], ps[:, :SC])

        qk_proj(wqt, xqt, qt)
        qk_proj(wkt, xkt, kt)

        # v projection: v[s, dv] natural orientation, scattered into va
        wv = wpool.tile([P, KO, HD], FR, tag="w", name="w_v")
        nc.sync.dma_start(wv, wvt.rearrange("(o p) m -> p o m", p=P))
        for c in range(n_sc):
            xv = xpool.tile([P, KO, SC], FR, tag="x", name="x_v")
            nc.sync.dma_start(
                xv, xvt[:, c * SC : (c + 1) * SC].rearrange("(o p) s -> p o s", p=P)
            )
            for st in range(SC // P):
                s_tile = c * (SC // P) + st
                ps = ps_small.tile([P, 512], F32, tag="ps", name="ps_v")
                for k in range(KO):
                    nc.tensor.matmul(
                        ps,
                        lhsT=xv[:, k, st * P : (st + 1) * P],
                        rhs=wv[:, k, :],
                        start=(k == 0),
                        stop=(k == KO - 1),
                    )
                nc.vector.tensor_copy(
                    va[:, s_tile, :, 0:DK],
                    ps.rearrange("p (h d) -> p h d", d=DK),
                )

        ps1_ctx.close()

        # ---------------- Phase 2: attention, head pairs ----------------
        # Heads 2*pr (rows 0:64, "A") and 2*pr+1 (rows 64:128, "B") run as
        # row-tiled concurrent score matmuls; one [128, 2*JC] exp covers both.
        JC = 512  # sq chunk
        n_c = S // JC
        ps2_ctx = ExitStack()
        ps_score = ps2_ctx.enter_context(tc.tile_pool(name="ps_score", bufs=2, space="PSUM"))
        ps_out = ps2_ctx.enter_context(tc.tile_pool(name="ps_out", bufs=2, space="PSUM"))
        for pr in range(4):
            qa, qb = qt[0:DK, pr, :], qt[DK:P, pr, :]
            ka, kb = kt[0:DK, pr, :], kt[DK:P, pr, :]
            ha, hb = 2 * pr, 2 * pr + 1
            for c in range(n_c):
                cs = slice(c * JC, (c + 1) * JC)
                po = ps_out.tile([65, 2 * JC], F32, tag="po", name="po")
                for sk in range(n_sk):
                    ks = slice(sk * P, (sk + 1) * P)
                    pss = ps_score.tile([P, 2 * JC], F32, tag="pss", name="pss")
                    nc.tensor.matmul(pss[:, 0:JC], lhsT=ka[:, ks], rhs=qa[:, cs],
                                     start=True, stop=True)
                    nc.tensor.matmul(pss[:, JC : 2 * JC], lhsT=kb[:, ks], rhs=qb[:, cs],
                                     start=True, stop=True)
                    ex = epool.tile([P, 2 * JC], BF, tag="ex", name="ex")
                    nc.scalar.activation(ex, pss, EXP, scale=0.125)
                    nc.tensor.matmul(po[:, 0:JC], lhsT=va[:, sk, ha, :], rhs=ex[:, 0:JC],
                                     start=(sk == 0), stop=(sk == n_sk - 1))
                    nc.tensor.matmul(po[:, JC : 2 * JC], lhsT=va[:, sk, hb, :],
                                     rhs=ex[:, JC : 2 * JC],
                                     start=(sk == 0), stop=(sk == n_sk - 1))
                # normalize both heads of the pair
                rr = rpool.tile([65, 2 * JC], F32, tag="rr", name="rr")
                nc.vector.reciprocal(rr[64:65, :], po[64:65, :])
                r0 = rpool.tile([1, 2 * JC], F32, tag="r0", name="r0")
                nc.sync.dma_start(r0, rr[64:65, :])
                bc = bpool.tile([DK, 2 * JC], F32, tag="bc", name="bc")
                nc.gpsimd.partition_broadcast(bc, r0, channels=DK)
                nc.vector.tensor_mul(at[0:DK, pr, cs], po[0:DK, 0:JC], bc[:, 0:JC])
                tt = tpool.tile([DK, JC], FR, tag="tt", name="tt")
                nc.vector.tensor_mul(tt, po[0:DK, JC : 2 * JC], bc[:, JC : 2 * JC])
                nc.sync.dma_start(at[DK:P, pr, cs], tt)

        ps2_ctx.close()

        # ---------------- Phase 3: output projection ----------------
        ps3_ctx = ExitStack()
        ps_small = ps3_ctx.enter_context(tc.tile_pool(name="ps_small3", bufs=4, space="PSUM"))
        wo = wpool.tile([P, 4, D], FR, tag="w", name="w_o")
        nc.sync.dma_start(wo, wot.rearrange("(pr p) n -> p pr n", p=P))
        for st in range(n_st):
            ps0 = ps_small.tile([P, 512], F32, tag="ps", name="ps_o0")
            ps1 = ps_small.tile([P, 512], F32, tag="ps", name="ps_o1")
            for pr in range(4):
                lhs = at[:, pr, st * P : (st + 1) * P]
                nc.tensor.matmul(
                    ps0, lhsT=lhs, rhs=wo[:, pr, 0:512],
                    start=(pr == 0), stop=(pr == 3),
                )
                nc.tensor.matmul(
                    ps1, lhsT=lhs, rhs=wo[:, pr, 512:1024],
                    start=(pr == 0), stop=(pr == 3),
                )
            ob0 = opool.tile([P, 512], F32, tag="ob", name="ob0")
            nc.vector.tensor_copy(ob0, ps0)
            nc.sync.dma_start(out[st * P : (st + 1) * P, 0:512], ob0)
            ob1 = opool.tile([P, 512], F32, tag="ob", name="ob1")
            nc.scalar.copy(ob1, ps1)
            nc.sync.dma_start(out[st * P : (st + 1) * P, 512:1024], ob1)
        ps3_ctx.close()

    nc.compile()
    return nc


_nc_cache = {}


def _get_nc(S=2048):
    if S not in _nc_cache:
        _nc_cache[S] = build(S)
    return _nc_cache[S]


def make_in_maps(Q, K, V, W_q, W_k, W_v, W_o):
    Q, K, V = (np.asarray(t, dtype=np.float32) for t in (Q, K, V))
    W_q, W_k, W_v, W_o = (np.asarray(t, dtype=np.float32) for t in (W_q, W_k, W_v, W_o))
    in_maps = []
    for c in range(8):
        b, hb = c // 2, c % 2
        sl = slice(hb * HD, (hb + 1) * HD)
        in_maps.append(
            {
                "xqt": np.ascontiguousarray(Q[b].T),
                "xkt": np.ascontiguousarray(K[b].T),
                "xvt": np.ascontiguousarray(V[b].T),
                "wqt": np.ascontiguousarray(W_q[sl, :].T),
                "wkt": np.ascontiguousarray(W_k[sl, :].T),
                "wvt": np.ascontiguousarray(W_v[sl, :].T),
                "wot": np.ascontiguousarray(W_o[:, sl].T),
            }
        )
    return in_maps


def kernel(Q, K, V, W_q, W_k, W_v, W_o):
    nc = _get_nc(2048)
    in_maps = make_in_maps(Q, K, V, W_q, W_k, W_v, W_o)
    res = run_bass_kernel_spmd(nc, in_maps, core_ids=list(range(8)))
    outs = [res.results[c]["out"] for c in range(8)]
    full = np.stack([outs[2 * b] + outs[2 * b + 1] for b in range(4)], axis=0)
    return full.astype(np.float32)



# revision 3
# speedup vs baseline: 1.0675x; 1.0675x over previous
"""Trainium2 Bass kernel for nn_MultiHeadAttention (B=4, S=2048, D=1024, H=16).

Sharding: 8 cores = (batch b in 0..3) x (head-half hb in 0..1).
Each core computes, for its batch b and its 8 heads:
  qT = (Q[b] @ W_q[hb].T).T        [512, S]   (features on partitions)
  kT = (K[b] @ W_k[hb].T).T        [512, S]
  v  =  V[b] @ W_v[hb].T           [S, 512]   (+ ones column per head)
  per head pair: scoresT -> exp -> PV (ones-row gives softmax denominator
  in the same PSUM accumulation), normalize, then out-projection partial.
Host sums the two partial outputs per batch.

v2: one software-pipelined schedule instead of three serial phases.
  - prefix: K-proj, V-proj, Q-proj(c0,p0) on PE while DMA streams inputs
  - main: 16 chunks (sq-chunk-major x head-pair). Per sk step: score
    matmuls feed the ACT engine's exp; PV matmuls lag LAG steps behind so
    the PE never waits on exp. JIT Q-proj units and output-projection
    units are interleaved as PE filler, keeping the PE continuously busy
    (full pstate) and the ACT engine saturated.
  - normalization: PSUM drained to SBUF immediately (frees the single po
    bank pair), reciprocal_approx_fast instead of the 6.5us full-precision
    reciprocal, broadcast + muls on the idle GPSIMD engine.
  - all matmul inputs in bf16 (halves DMA; full PE rate).
"""

import sys

sys.path.insert(0, "/opt/trn_rl_repo")

from contextlib import ExitStack

import ml_dtypes
import numpy as np

import concourse.bass as bass  # noqa: F401
import concourse.tile as tile
from concourse import bacc, mybir
from concourse.bass_utils import run_bass_kernel_spmd

F32 = mybir.dt.float32
BF = mybir.dt.bfloat16
EXP = mybir.ActivationFunctionType.Exp

D = 1024  # d_model
HD = 512  # head-dim slice per core (8 heads x 64)
DK = 64
NH = 8  # heads per core
P = 128
S = 2048
SC = 512  # proj streaming chunk of S
JC = 512  # sq chunk per head in attention
KO = D // P  # 8 contraction subtiles
N_SK = S // P  # 16 sk tiles
N_C = S // SC  # 4
LAG = 3  # PV lags exp by this many sk steps


def build():
    nc = bacc.Bacc(
        "TRN2",
        target_bir_lowering=False,
        debug=False,
        enable_asserts=False,
        num_devices=1,
    )

    xqt = nc.dram_tensor("xqt", [D, S], BF, kind="ExternalInput").ap()
    xkt = nc.dram_tensor("xkt", [D, S], BF, kind="ExternalInput").ap()
    xvt = nc.dram_tensor("xvt", [D, S], BF, kind="ExternalInput").ap()
    wqt = nc.dram_tensor("wqt", [D, HD], BF, kind="ExternalInput").ap()
    wkt = nc.dram_tensor("wkt", [D, HD], BF, kind="ExternalInput").ap()
    wvt = nc.dram_tensor("wvt", [D, HD], BF, kind="ExternalInput").ap()
    wot = nc.dram_tensor("wot", [HD, D], BF, kind="ExternalInput").ap()
    out = nc.dram_tensor("out", [S, D], F32, kind="ExternalOutput").ap()

    with tile.TileContext(nc) as tc, ExitStack() as ctx:
        pers = ctx.enter_context(tc.tile_pool(name="pers", bufs=1))
        wpool = ctx.enter_context(tc.tile_pool(name="wpool", bufs=3))
        xpool = ctx.enter_context(tc.tile_pool(name="xpool", bufs=2))
        epool = ctx.enter_context(tc.tile_pool(name="epool", bufs=6))
        pupool = ctx.enter_context(tc.tile_pool(name="pupool", bufs=2))
        rpool = ctx.enter_context(tc.tile_pool(name="rpool", bufs=2))
        bpool = ctx.enter_context(tc.tile_pool(name="bpool", bufs=2))
        tpool = ctx.enter_context(tc.tile_pool(name="tpool", bufs=2))
        opool = ctx.enter_context(tc.tile_pool(name="opool", bufs=4))
        ps_score = ctx.enter_context(tc.tile_pool(name="ps_score", bufs=2, space="PSUM"))
        ps_out = ctx.enter_context(tc.tile_pool(name="ps_out", bufs=1, space="PSUM"))
        ps_small = ctx.enter_context(tc.tile_pool(name="ps_small", bufs=2, space="PSUM"))

        # persistent intermediates
        qt = pers.tile([P, 4, S], BF)  # qT: [p, pair, s], feature = pair*128+p
        kt = pers.tile([P, 4, S], BF)
        va = pers.tile([P, N_SK, NH, DK + 1], BF)  # [s%128, s//128, h, dv|1]
        at = pers.tile([P, 4, S], BF)  # attnT (normalized)

        nc.vector.memset(va[:, :, :, DK], 1.0)

        def load_x(src, c, nm):
            x = xpool.tile([P, KO, SC], BF, tag="x", name=nm)
            nc.sync.dma_start(
                x, src[:, c * SC : (c + 1) * SC].rearrange("(o p) s -> p o s", p=P)
            )
            return x

        def qk_unit(w, x, dst, pr):
            ps = ps_small.tile([P, SC], F32, tag="ps", name="ps_qk")
            for k in range(KO):
                nc.tensor.matmul(
                    ps,
                    lhsT=w[:, k, pr * P : (pr + 1) * P],
                    rhs=x[:, k, :],
                    start=(k == 0),
                    stop=(k == KO - 1),
                )
            nc.vector.tensor_copy(dst, ps)

        def v_unit(x, c, st):
            ps = ps_small.tile([P, HD], F32, tag="ps", name="ps_v")
            for k in range(KO):
                nc.tensor.matmul(
                    ps,
                    lhsT=x[:, k, st * P : (st + 1) * P],
                    rhs=wv[:, k, :],
                    start=(k == 0),
                    stop=(k == KO - 1),
                )
            nc.vector.tensor_copy(
                va[:, c * (SC // P) + st, :, 0:DK],
                ps.rearrange("p (h d) -> p h d", d=DK),
            )

        def p3_unit(st, half):
            ps = ps_small.tile([P, 512], F32, tag="ps", name="ps_o")
            for pr in range(4):
                nc.tensor.matmul(
                    ps,
                    lhsT=at[:, pr, st * P : (st + 1) * P],
                    rhs=wo[:, pr, half * 512 : (half + 1) * 512],
                    start=(pr == 0),
                    stop=(pr == 3),
                )
            ob = opool.tile([P, 512], F32, tag="ob", name="ob")
            nc.vector.tensor_copy(ob, ps)
            nc.sync.dma_start(out[st * P : (st + 1) * P, half * 512 : (half + 1) * 512], ob)

        # ---------------- prefix: K-proj, V-proj, Q-proj(c0,p0) ----------------
        # bufs=2 x-pool + concurrent DMA engine gives one-chunk-deep prefetch:
        # the dma for chunk c only WAR-waits on readers of chunk c-2.
        wk = wpool.tile([P, KO, HD], BF, tag="w", name="wk")
        nc.sync.dma_start(wk, wkt.rearrange("(o p) m -> p o m", p=P))
        for c in range(N_C):
            xk = load_x(xkt, c, f"xk{c}")
            for pr in range(4):
                qk_unit(wk, xk, kt[:, pr, c * SC : (c + 1) * SC], pr)
        wv = wpool.tile([P, KO, HD], BF, tag="w", name="wv")
        nc.sync.dma_start(wv, wvt.rearrange("(o p) m -> p o m", p=P))
        wq = wpool.tile([P, KO, HD], BF, tag="w", name="wq")
        nc.sync.dma_start(wq, wqt.rearrange("(o p) m -> p o m", p=P))
        for c in range(N_C):
            xv = load_x(xvt, c, f"xv{c}")
            for st in range(SC // P):
                v_unit(xv, c, st)
        xq = {0: load_x(xqt, 0, "xq0")}
        wo = wpool.tile([P, 4, D], BF, tag="w", name="wo")
        nc.sync.dma_start(wo, wot.rearrange("(pr p) n -> p pr n", p=P))
        qdone = set()

        def q_unit(c, pr):
            qk_unit(wq, xq[c], qt[:, pr, c * SC : (c + 1) * SC], pr)
            qdone.add((c, pr))

        q_unit(0, 0)

        # ---------------- main pipelined attention loop ----------------
        exq = []  # FIFO of (ex, sk, meta); meta = [po, pi, cs]
        p3_backlog = []

        def norm(meta):
            po, pi, cs = meta
            pu = pupool.tile([65, 2 * JC], F32, tag="pu", name="pu")
            nc.vector.tensor_copy(pu, po)  # frees the po bank pair
            rb = rpool.tile([1, 2 * JC], F32, tag="rb", name="rb")
            nc.sync.dma_start(rb, pu[64:65, :])
            rc = rpool.tile([1, 2 * JC], F32, tag="rc", name="rc")
            nc.vector.reciprocal_approx_fast(out=rc, in_=rb)
            bc = bpool.tile([DK, 2 * JC], F32, tag="bc", name="bc")
            nc.gpsimd.partition_broadcast(bc, rc, channels=DK)
            nc.gpsimd.tensor_mul(at[0:DK, pi, cs], pu[0:DK, 0:JC], bc[:, 0:JC])
            tt = tpool.tile([DK, JC], BF, tag="tt", name="tt")
            nc.gpsimd.tensor_mul(tt, pu[0:DK, JC : 2 * JC], bc[:, JC : 2 * JC])
            nc.sync.dma_start(at[DK:P, pi, cs], tt)

        def pv_step():
            ex, sk, meta = exq.pop(0)
            if sk == 0:
                meta[0] = ps_out.tile([65, 2 * JC], F32, tag="po", name="po")
            po, pi, cs = meta
            ha, hb = 2 * pi, 2 * pi + 1
            last = sk == N_SK - 1
            nc.tensor.matmul(
                po[:, 0:JC], lhsT=va[:, sk, ha, :], rhs=ex[:, 0:JC],
                start=(sk == 0), stop=last,
            )
            nc.tensor.matmul(
                po[:, JC : 2 * JC], lhsT=va[:, sk, hb, :], rhs=ex[:, JC : 2 * JC],
                start=(sk == 0), stop=last,
            )
            if last:
                norm(meta)
                if pi == 3:
                    ci = cs.start // JC
                    for st in range(4 * ci, 4 * ci + 4):
                        p3_backlog.append((st, 0))
                        p3_backlog.append((st, 1))

        for ci in range(N_C):
            for pi in range(4):
                cs = slice(ci * JC, (ci + 1) * JC)
                meta = [None, pi, cs]
                for sk in range(N_SK):
                    if sk == 2:
                        nci, npi = (ci, pi + 1) if pi < 3 else (ci + 1, 0)
                        if nci < N_C and (nci, npi) not in qdone:
                            q_unit(nci, npi)
                    if sk == 7 and pi == 2 and ci + 1 < N_C and ci + 1 not in xq:
                        xq[ci + 1] = load_x(xqt, ci + 1, f"xq{ci + 1}")
                    if sk in (5, 9, 13) and pi >= 1 and p3_backlog:
                        p3_unit(*p3_backlog.pop(0))
                    pss = ps_score.tile([P, 2 * JC], F32, tag="pss", name="pss")
                    ks = slice(sk * P, (sk + 1) * P)
                    nc.tensor.matmul(
                        pss[:, 0:JC], lhsT=kt[0:DK, pi, ks], rhs=qt[0:DK, pi, cs],
                        start=True, stop=True,
                    )
                    nc.tensor.matmul(
                        pss[:, JC : 2 * JC], lhsT=kt[DK:P, pi, ks], rhs=qt[DK:P, pi, cs],
                        start=True, stop=True,
                    )
                    ex = epool.tile([P, 2 * JC], BF, tag="ex", name="ex")
                    nc.scalar.activation(ex, pss, EXP, scale=0.125)
                    exq.append((ex, sk, meta))
                    if len(exq) > LAG:
                        pv_step()

        while exq:
            pv_step()
        while p3_backlog:
            p3_unit(*p3_backlog.pop(0))

    nc.compile()
    return nc


_nc_cache = {}


def _get_nc(S_=2048):
    if S_ not in _nc_cache:
        _nc_cache[S_] = build()
    return _nc_cache[S_]


def _bf(a):
    return np.ascontiguousarray(a).astype(ml_dtypes.bfloat16)


def make_in_maps(Q, K, V, W_q, W_k, W_v, W_o):
    Q, K, V = (np.asarray(t, dtype=np.float32) for t in (Q, K, V))
    W_q, W_k, W_v, W_o = (np.asarray(t, dtype=np.float32) for t in (W_q, W_k, W_v, W_o))
    in_maps = []
    for c in range(8):
        b, hb = c // 2, c % 2
        sl = slice(hb * HD, (hb + 1) * HD)
        in_maps.append(
            {
                "xqt": _bf(Q[b].T),
                "xkt": _bf(K[b].T),
                "xvt": _bf(V[b].T),
                "wqt": _bf(W_q[sl, :].T),
                "wkt": _bf(W_k[sl, :].T),
                "wvt": _bf(W_v[sl, :].T),
                "wot": _bf(W_o[:, sl].T),
            }
        )
    return in_maps


def kernel(Q, K, V, W_q, W_k, W_v, W_o):
    nc = _get_nc(2048)
    in_maps = make_in_maps(Q, K, V, W_q, W_k, W_v, W_o)
    res = run_bass_kernel_spmd(nc, in_maps, core_ids=list(range(8)))
    outs = [res.results[c]["out"] for c in range(8)]
    full = np.stack([outs[2 * b] + outs[2 * b + 1] for b in range(4)], axis=0)
    return full.astype(np.float32)
